# revision 3
# baseline (speedup 1.0000x reference)
"""Att-BiLSTM-CRF Trainium2 kernel, v2.

Data-parallel over batch: 32 seqs -> 8 cores x 4 (BL=4). All-f32 device math.

Device pipeline per core:
  scan1   chunked BiLSTM-1: P1=4 chunks/dir x (128+64) lockstep slots.
          Warmup slots (64) converge to the exact trajectory via forget-gate
          damping (validated: max |dh| <= 4e-4, zero Viterbi flips).
          Per slot/dir: x-part matmuls (W1@emb + bias K=1 ones-matmul)
          accumulate in PSUM, 16 Whh@h matmuls on top, ONE tanh gates-ACT
          (sigmoid via tanh(z/2), scales folded into weights host-side),
          3 fused scalar_tensor_tensor cell ops, h* = 2h carried (0.5
          absorbed into consumer weights), hist writes on gpsimd.
  phaseC  attention + X2 -> x2 gate preacts staged in DRAM, s-blocks of 64
          ordered {0,7,1,6,...} (edges first). Two blocks lead-in; the rest
          interleave into scan2's idle engine slots (emitted early enough in
          program order that every x2 write precedes its read).
  scan2   sequential 512 steps (layer-2 forget gates saturate; chunking
          provably does not converge). Same fused step structure; x2 enters
          PSUM via identity-matmul from 8-step DMA batches.
  phaseD  feats = h2t@l2 + bias, blocks interleaved middle-out into scan2.
host: embedding gather before; CRF Viterbi decode after (numpy).
"""

import numpy as np

S = 512
D = 256
H = 256
G = 4 * H
T = 12
NS = 16
B = 32
NCORES = 8
BL = B // NCORES
PAD, START, STOP = 0, 10, 11
N_ALL = S * BL

P1 = 4                    # scan1 chunks per direction
L1 = S // P1              # 128
D1 = 64                   # warmup slots
NSLOT = L1 + D1           # 192
NV = 640                  # padded virtual emb positions: [64 zero | 512 real | 64 zero]

NBLK = 8
BS = S // NBLK            # 64
BLK_ORDER = [0, 7, 1, 6, 2, 5, 3, 4]

PERM = [0, 1, 2, 3, 6, 7, 4, 5]  # m-tiles -> [i0,i1,f0,f1,o0,o1,g0,g1]

_BUILT = None
DEBUG = False
F32R_X2 = True
F32R_WX = False
F32R_FEATS = False


def _reorder_rows(w):
    wt = w.reshape(8, 128, -1)
    return wt[PERM].reshape(G, -1)


def _lhsT_layout(w, kchunks):
    """w: (G, K) -> (128, kchunks, 8, 128): [p,c,m,q] = w[m*128+q, c*128+p]."""
    wr = _reorder_rows(np.asarray(w, np.float32))
    K = wr.shape[1]
    assert K == kchunks * 128
    a = wr.T.reshape(kchunks, 128, 8, 128)
    return np.ascontiguousarray(a.transpose(1, 0, 2, 3)).astype(np.float32)


def _scale_gates(w, s_ifo, s_g):
    """Scale gate rows of (G, ...): i,f,o rows by s_ifo, g rows by s_g.
    Reference gate order along G is i,f,g,o (each H wide)."""
    w = np.asarray(w, np.float64).copy()
    w[0 * H:2 * H] *= s_ifo   # i, f
    w[2 * H:3 * H] *= s_g     # g
    w[3 * H:4 * H] *= s_ifo   # o
    return w.astype(np.float32)


def _build():
    import concourse.tile as tile
    from concourse.bacc import Bacc
    from concourse import mybir

    f32 = mybir.dt.float32
    f32r = mybir.dt.float32r
    AF = mybir.ActivationFunctionType
    ALU = mybir.AluOpType

    nc = Bacc()
    h1dt = f32r if F32R_WX else f32
    h2dt = f32r if F32R_FEATS else f32
    w2dt = f32r if F32R_X2 else f32

    def din(name, shape, dt=None):
        return nc.dram_tensor(name, shape, dt or f32, kind="ExternalInput")

    embP = din("embP", (128, 2, NV * BL))          # [p, kc, v*BL+b]
    w1T = din("w1T", (128, 2, 2, 8, 128))          # [p, dir, kc, m, q]
    whh1T = din("whh1T", (128, 2, 2, 8, 128))
    bf16 = mybir.dt.bfloat16
    b1hl = nc.dram_tensor("b1hl", (8, 2, 2, 128), bf16, kind="ExternalInput")
    oneh = nc.dram_tensor("oneh", (8, 8, P1 * BL), bf16, kind="ExternalInput")
    whh2T = din("whh2T", (128, 2, 2, 8, 128))
    w2T = din("w2T", (128, 2, 8, 8, 128), w2dt)
    b2col = din("b2col", (128, 2, 8))              # per-partition bias2 [p,d,m]
    attWT = din("attWT", (128, 4, 4, 128), h1dt)   # [p, kc, f, q] (x0.5 folded)
    sentT = din("sentT", (128, BL, 4, NS))
    sentN = din("sentN", (NS, BL, 2 * H))
    h2tT = din("h2tT", (128, 2, 2, T), h2dt)       # [p, dir, kc, t] (x0.5)
    h2tb = din("h2tb", (T, 1))
    identD = din("ident", (128, 128))

    scratch_kind = "ExternalOutput" if DEBUG else "Internal"
    x2d = nc.dram_tensor("x2d", (2, NBLK, 128, BS, 8, BL), f32,
                         kind=scratch_kind)
    if DEBUG:
        h1d = nc.dram_tensor("h1d", (2, 128, 2, N_ALL), h1dt,
                             kind="ExternalOutput")
        gtd = nc.dram_tensor("gtd", (128, 8, P1, BL), f32,
                             kind="ExternalOutput")
        ptd = nc.dram_tensor("ptd", (128, 8, P1, BL), f32,
                             kind="ExternalOutput")
        hsd = nc.dram_tensor("hsd", (128, 2, P1, BL), f32,
                             kind="ExternalOutput")
        wsd = nc.dram_tensor("wsd", (128, 2, P1, BL), f32,
                             kind="ExternalOutput")
        h2d = nc.dram_tensor("h2d", (2, 128, 2, N_ALL), h2dt,
                             kind="ExternalOutput")
    featsT = nc.dram_tensor("featsT", (T, N_ALL), f32, kind="ExternalOutput")

    with tile.TileContext(nc) as tc:
        with tc.tile_pool(name="persist", bufs=1) as pp:
            hist1 = [pp.tile([128, 2, N_ALL], h1dt, tag=f"h1_{d}", name=f"h1_{d}")
                     for d in range(2)]
            hist2 = [pp.tile([128, 2, N_ALL], h2dt, tag=f"h2_{d}", name=f"h2_{d}")
                     for d in range(2)]
            ident_sb = pp.tile([128, 128], f32)
            nc.sync.dma_start(out=ident_sb, in_=identD[:])
            b1hl_sb = pp.tile([8, 2, 2, 128], bf16)
            nc.sync.dma_start(out=b1hl_sb, in_=b1hl[:])
            oneh_sb = pp.tile([8, 8, P1 * BL], bf16)
            nc.sync.dma_start(out=oneh_sb, in_=oneh[:])
            ones16 = pp.tile([NS, NS], f32)
            nc.vector.memset(ones16, 1.0)
            ones_cell = pp.tile([128, 2, P1, BL], f32)
            nc.vector.memset(ones_cell, 1.0)
            ones_w = pp.tile([128, BS * BL], f32)
            nc.vector.memset(ones_w, 1.0)

            def hist_k(hist, c):
                return hist[0][:, c, :] if c < 2 else hist[1][:, c - 2, :]

            # ======================= scan1 (chunked) =======================
            with tc.tile_pool(name="s1w", bufs=1) as s1w, \
                 tc.tile_pool(name="s1st", bufs=3) as s1st, \
                 tc.tile_pool(name="s1ps", bufs=4, space="PSUM") as s1ps:
                emb_sb = s1w.tile([128, 2, NV * BL], f32, tag="embP", name="embP")
                nc.sync.dma_start(out=emb_sb, in_=embP[:])
                w1_sb = s1w.tile([128, 2, 2, 8, 128], f32, tag="w1", name="w1")
                nc.sync.dma_start(out=w1_sb, in_=w1T[:])
                whh1_sb = s1w.tile([128, 2, 2, 8, 128], f32, tag="wh1", name="wh1")
                nc.sync.dma_start(out=whh1_sb, in_=whh1T[:])

                embv = emb_sb.rearrange("p c (v b) -> p c v b", b=BL)
                Hs = [s1w.tile([128, 2, P1, BL], f32, tag=f"H{d}", name=f"H{d}")
                      for d in range(2)]
                Ws = [s1w.tile([128, 2, P1, BL], f32, tag=f"W{d}", name=f"W{d}")
                      for d in range(2)]
                for d in range(2):
                    nc.vector.memset(Hs[d], 0.0)
                    nc.vector.memset(Ws[d], 0.0)
                h1r = [hist1[d].rearrange("p c (q r) -> p c q r", q=P1)
                       for d in range(2)]

                for k in range(NSLOT):
                    if k == D1:
                        # chunk with no real history restarts from true init:
                        # fwd chunk 0 (seq start), bwd chunk P1-1 (seq end)
                        nc.vector.memset(Hs[0][:, :, 0, :], 0.0)
                        nc.vector.memset(Ws[0][:, :, 0, :], 0.0)
                        nc.vector.memset(Hs[1][:, :, P1 - 1, :], 0.0)
                        nc.vector.memset(Ws[1][:, :, P1 - 1, :], 0.0)
                    for d in range(2):
                        # fwd: chunk q reads v = q*128 + k  (pos = v - 64)
                        # bwd: chunk q reads v = q*128 + 255 - k
                        vbase = k if d == 0 else (255 - k)
                        pt = s1ps.tile([128, 8, P1, BL], f32, tag=f"pt{d}")
                        for m in range(8):
                            for c in range(2):
                                # start=True pends the whole 2KB zero region;
                                # each region's first touch then overwrites.
                                nc.tensor.matmul(
                                    pt[:, m, :, :], w1_sb[:, d, c, m, :],
                                    embv[:, c, vbase::L1, :][:, 0:P1, :],
                                    start=(m == 0 and c == 0), stop=False)
                        for hl in range(2):
                            nc.tensor.matmul(
                                pt.rearrange("p m q b -> p (m q b)"),
                                b1hl_sb[:, hl, d, :], oneh_sb.rearrange(
                                    "k m j -> k (m j)"),
                                start=False, stop=False)
                        for m in range(8):
                            for c in range(2):
                                nc.tensor.matmul(
                                    pt[:, m, :, :], whh1_sb[:, d, c, m, :],
                                    Hs[d][:, c, :, :],
                                    start=False, stop=(c == 1 and m == 7))
                        gt = s1st.tile([128, 8, P1, BL], f32, tag=f"g{d}")
                        if DEBUG and k == D1 and d == 0:
                            ptsb = s1st.tile([128, 8, P1, BL], f32, tag="ptdbg")
                            nc.vector.tensor_copy(ptsb, pt)
                            nc.sync.dma_start(out=ptd[:], in_=ptsb)
                        nc.scalar.activation(out=gt, in_=pt, func=AF.Tanh,
                                             scale=0.5)
                        if DEBUG and k == D1 and d == 0:
                            nc.sync.dma_start(out=gtd[:], in_=gt)
                        pv = s1st.tile([128, 2, P1, BL], f32, tag=f"p{d}")
                        nc.vector.scalar_tensor_tensor(
                            out=pv, in0=gt[:, 2:4], scalar=1.0, in1=Ws[d],
                            op0=ALU.add, op1=ALU.mult)
                        vv = s1st.tile([128, 2, P1, BL], f32, tag=f"v{d}")
                        nc.vector.scalar_tensor_tensor(
                            out=vv, in0=gt[:, 0:2], scalar=1.0, in1=gt[:, 6:8],
                            op0=ALU.add, op1=ALU.mult)
                        nc.vector.scalar_tensor_tensor(
                            out=Ws[d], in0=pv, scalar=0.5, in1=vv,
                            op0=ALU.mult, op1=ALU.add)
                        th = s1st.tile([128, 2, P1, BL], f32, tag=f"t{d}")
                        nc.scalar.activation(out=th, in_=Ws[d], func=AF.Tanh,
                                             scale=0.5)
                        nc.vector.scalar_tensor_tensor(
                            out=Hs[d], in0=gt[:, 4:6], scalar=1.0, in1=th,
                            op0=ALU.add, op1=ALU.mult)
                        if DEBUG and k == D1 and d == 0:
                            nc.sync.dma_start(out=hsd[:], in_=Hs[0])
                            nc.sync.dma_start(out=wsd[:], in_=Ws[0])
                        if k >= D1:
                            j = k - D1
                            col = j if d == 0 else (L1 - 1 - j)
                            nc.gpsimd.tensor_tensor(
                                out=h1r[d][:, :, :, BL * col:BL * (col + 1)],
                                in0=Hs[d], in1=ones_cell, op=ALU.mult)

            if DEBUG:
                for d in range(2):
                    nc.sync.dma_start(out=h1d[d], in_=hist1[d])

            # =================== phase C + scan2 + phase D ==================
            with tc.tile_pool(name="p2w", bufs=1) as p2w, \
                 tc.tile_pool(name="p2c", bufs=2) as p2c, \
                 tc.tile_pool(name="p2s", bufs=3) as p2s, \
                 tc.tile_pool(name="x2p", bufs=2) as x2p, \
                 tc.tile_pool(name="stgp", bufs=1) as stgp, \
                 tc.tile_pool(name="cmbp", bufs=1) as cmbp, \
                 tc.tile_pool(name="aps", bufs=2, space="PSUM") as aps, \
                 tc.tile_pool(name="s2ps", bufs=2, space="PSUM") as s2ps:
                attW_sb = p2w.tile([128, 4, 4, 128], h1dt, tag="aW", name="aW")
                nc.sync.dma_start(out=attW_sb, in_=attWT[:])
                sentT_sb = p2w.tile([128, BL, 4, NS], f32, tag="sT", name="sT")
                nc.sync.dma_start(out=sentT_sb, in_=sentT[:])
                sentN_sb = p2w.tile([NS, BL, 2 * H], f32, tag="sN", name="sN")
                nc.sync.dma_start(out=sentN_sb, in_=sentN[:])
                w2_sb = p2w.tile([128, 2, 8, 8, 128], w2dt, tag="w2", name="w2")
                nc.sync.dma_start(out=w2_sb, in_=w2T[:])
                whh2_sb = p2w.tile([128, 2, 2, 8, 128], f32, tag="wh2", name="wh2")
                nc.sync.dma_start(out=whh2_sb, in_=whh2T[:])
                b2_sb = p2w.tile([128, 2, 8], f32, tag="b2", name="b2")
                nc.sync.dma_start(out=b2_sb, in_=b2col[:])
                h2t_sb = p2w.tile([128, 2, 2, T], h2dt, tag="h2t", name="h2t")
                nc.sync.dma_start(out=h2t_sb, in_=h2tT[:])
                h2tb_sb = p2w.tile([T, 1], f32, tag="h2tb", name="h2tb")
                nc.sync.dma_start(out=h2tb_sb, in_=h2tb[:])

                def phasec_block(pieces):
                    """Attention + X2 for a list of (blk, s0, ns) pieces
                    (total width 64 positions = 256 n, so the X2 matmuls hit
                    the f32r fast-rate threshold). Yields per ~PE quantum."""
                    assert sum(ns for _, _, ns in pieces) == BS
                    cmb = cmbp.tile([128, 4, BS * BL], f32, tag="cmb")
                    cmbR = cmbp.tile([128, 8, BS * BL],
                                     f32r if F32R_X2 else f32, tag="cmbR",
                                     name="cmbR")
                    cmbv = cmb.rearrange("p f (s b) -> p f s b", b=BL)
                    cmbRv = cmbR.rearrange("p f (s b) -> p f s b", b=BL)
                    col = 0
                    for blk, s0, ns in pieces:
                        n0 = (blk * BS + s0) * BL
                        NW = ns * BL
                        c0 = col * BL
                        for f in range(4):
                            pwx_t = aps.tile([128, BS * BL], f32, tag="big")
                            pwx = pwx_t[:, 0:NW]
                            for c in range(4):
                                nc.tensor.matmul(
                                    pwx, attW_sb[:, c, f, :],
                                    hist_k(hist1, c)[:, n0:n0 + NW],
                                    start=(c == 0), stop=(c == 3))
                                if c == 1 and ns > 16:
                                    yield
                            nc.vector.tensor_copy(cmb[:, f, c0:c0 + NW], pwx)
                            nc.gpsimd.tensor_tensor(
                                out=cmbR[:, f, c0:c0 + NW],
                                in0=cmb[:, f, c0:c0 + NW],
                                in1=ones_w[:, 0:NW], op=ALU.mult)
                            yield
                        for b in range(BL):
                            ps16 = aps.tile([128, BS], f32, tag="sml")
                            for c in range(4):
                                nc.tensor.matmul(
                                    ps16[0:NS, 0:ns], sentT_sb[:, b, c, :],
                                    cmbv[:, c, col:col + ns, b],
                                    start=(c == 0), stop=(c == 3))
                            et = p2c.tile([NS, BS], f32, tag="et")
                            nc.scalar.activation(out=et[:, 0:ns],
                                                 in_=ps16[0:NS, 0:ns],
                                                 func=AF.Exp)
                            psum = aps.tile([128, BS], f32, tag="sml")
                            nc.tensor.matmul(psum[0:NS, 0:ns], ones16,
                                             et[:, 0:ns], start=True, stop=True)
                            rs = p2c.tile([NS, BS], f32, tag="rs")
                            nc.vector.reciprocal(out=rs[:, 0:ns],
                                                 in_=psum[0:NS, 0:ns])
                            aw = p2c.tile([NS, BS], f32, tag="aw")
                            nc.vector.tensor_mul(out=aw[:, 0:ns],
                                                 in0=et[:, 0:ns],
                                                 in1=rs[:, 0:ns])
                            yield
                            for f in range(4):
                                pg = aps.tile([128, BS], f32, tag="sml")
                                nc.tensor.matmul(
                                    pg[:, 0:ns],
                                    sentN_sb[:, b, 128 * f:128 * (f + 1)],
                                    aw[:, 0:ns], start=True, stop=True)
                                nc.vector.tensor_copy(
                                    cmbRv[:, 4 + f, col:col + ns, b],
                                    pg[:, 0:ns])
                                if f % 2 == 1 and ns > 16:
                                    yield
                        col += ns
                    for d in range(2):
                        stg = stgp.tile([128, BS, 8, BL], f32, tag=f"sg{d}")
                        for m in range(8):
                            px_t = aps.tile([128, BS * BL], f32, tag="big")
                            px = px_t
                            for c in range(8):
                                nc.tensor.matmul(
                                    px, w2_sb[:, d, c, m, :], cmbR[:, c, :],
                                    start=(c == 0), stop=(c == 7))
                                if c % 2 == 1 and not F32R_X2:
                                    yield
                            nc.scalar.activation(
                                out=stg[:, :, m, :],
                                in_=px.rearrange("p (s b) -> p s b", b=BL),
                                func=AF.Identity,
                                bias=b2_sb[:, d, m:m + 1])
                            yield
                        col = 0
                        for blk, s0, ns in pieces:
                            nc.sync.dma_start(
                                out=x2d[d, blk, :, s0:s0 + ns],
                                in_=stg[:, col:col + ns])
                            col += ns
                        yield

                def feats_block(blk):
                    n0 = blk * BS * BL
                    pf = aps.tile([128, BS * BL], f32, tag="big")
                    first = True
                    for d in range(2):
                        for c in range(2):
                            nc.tensor.matmul(
                                pf[0:T, :], h2t_sb[:, d, c, :],
                                hist2[d][:, c, n0:n0 + BS * BL],
                                start=first, stop=(d == 1 and c == 1))
                            first = False
                    yield
                    ft = p2c.tile([T, BS * BL], f32, tag="ft")
                    nc.vector.tensor_scalar_add(out=ft, in0=pf[0:T, :],
                                                scalar1=h2tb_sb)
                    nc.sync.dma_start(out=featsT[:, n0:n0 + BS * BL], in_=ft)
                    yield

                # lead-in: first 32 positions of each sequence end
                for _ in phasec_block([(0, 0, 16), (7, 48, 16),
                                       (0, 16, 16), (7, 32, 16)]):
                    pass
                for _ in phasec_block([(0, 32, 16), (7, 16, 16),
                                       (0, 48, 16), (7, 0, 16)]):
                    pass

                # remaining phase C, deadline-paced into scan2 (deadline =
                # scan2 step by which the group must be fully EMITTED --
                # its x2 read DMA is issued at consumption_step - 8)
                side = []
                for dl, pieces in (
                        (54, [(1, 0, 32), (6, 32, 32)]),
                        (86, [(1, 32, 32), (6, 0, 32)]),
                        (118, [(2, 0, 64)]), (120, [(5, 0, 64)]),
                        (182, [(3, 0, 64)]), (184, [(4, 0, 64)])):
                    side.append((dl, phasec_block(pieces)))
                featsq = [(322, feats_block(3)), (324, feats_block(4)),
                          (386, feats_block(2)), (388, feats_block(5)),
                          (450, feats_block(1)), (452, feats_block(6))]

                def emit_side(t):
                    while side:
                        dl = side[0][0]
                        if dl <= t:
                            quota = 1000   # overdue: flush
                        elif dl - t < 16:
                            quota = 3
                        else:
                            quota = 2
                        for _ in range(quota):
                            try:
                                next(side[0][1])
                            except StopIteration:
                                side.pop(0)
                                break
                        else:
                            break
                        if not side or side[0][0] > t + 16:
                            break
                    while featsq and featsq[0][0] <= t:
                        _, gen = featsq[0]
                        done = True
                        for _ in gen:
                            done = False
                            break
                        if done:
                            featsq.pop(0)

                # ----------------------- scan2 ------------------------------
                H2 = [p2w.tile([128, 2, BL], f32, tag=f"H2{d}", name=f"H2{d}")
                      for d in range(2)]
                W2 = [p2w.tile([128, 2, BL], f32, tag=f"W2{d}", name=f"W2{d}")
                      for d in range(2)]
                for d in range(2):
                    nc.vector.memset(H2[d], 0.0)
                    nc.vector.memset(W2[d], 0.0)
                h2r = [hist2[d] for d in range(2)]
                xt = {}

                def load_batch(d, tb):
                    """DMA the 8-step x2 batch covering steps tb..tb+7."""
                    if d == 0:
                        blk, s0 = tb // BS, tb % BS
                    else:
                        s = S - 1 - tb
                        blk, s0 = s // BS, (s % BS) - 7
                    tl = x2p.tile([128, 8, 8, BL], f32, tag=f"x{d}")
                    nc.sync.dma_start(out=tl, in_=x2d[d, blk, :, s0:s0 + 8])
                    return tl

                for d in range(2):
                    xt[(d, 0)] = load_batch(d, 0)

                for t in range(S):
                    if t % 8 == 0 and t + 8 < S:
                        for d in range(2):
                            xt[(d, t + 8)] = load_batch(d, t + 8)
                    emit_side(t)
                    for d in range(2):
                        tb = t - t % 8
                        idx = (t % 8) if d == 0 else (7 - t % 8)
                        xtile = xt[(d, tb)]
                        pt = s2ps.tile([128, 8, BL], f32, tag=f"q{d}")
                        nc.tensor.matmul(
                            pt, ident_sb, xtile[:, idx, :, :],
                            start=True, stop=False)
                        for m in range(8):
                            for c in range(2):
                                nc.tensor.matmul(
                                    pt[:, m, :], whh2_sb[:, d, c, m, :],
                                    H2[d][:, c, :],
                                    start=False, stop=(c == 1 and m == 7))
                        gt = p2s.tile([128, 8, BL], f32, tag=f"G{d}")
                        nc.scalar.activation(out=gt, in_=pt, func=AF.Tanh,
                                             scale=0.5)
                        pv = p2s.tile([128, 2, BL], f32, tag=f"P{d}")
                        nc.vector.scalar_tensor_tensor(
                            out=pv, in0=gt[:, 2:4], scalar=1.0, in1=W2[d],
                            op0=ALU.add, op1=ALU.mult)
                        vv = p2s.tile([128, 2, BL], f32, tag=f"V{d}")
                        nc.vector.scalar_tensor_tensor(
                            out=vv, in0=gt[:, 0:2], scalar=1.0, in1=gt[:, 6:8],
                            op0=ALU.add, op1=ALU.mult)
                        nc.vector.scalar_tensor_tensor(
                            out=W2[d], in0=pv, scalar=0.5, in1=vv,
                            op0=ALU.mult, op1=ALU.add)
                        th = p2s.tile([128, 2, BL], f32, tag=f"T{d}")
                        nc.scalar.activation(out=th, in_=W2[d], func=AF.Tanh,
                                             scale=0.5)
                        nc.vector.scalar_tensor_tensor(
                            out=H2[d], in0=gt[:, 4:6], scalar=1.0, in1=th,
                            op0=ALU.add, op1=ALU.mult)
                        s = t if d == 0 else S - 1 - t
                        nc.gpsimd.tensor_tensor(
                            out=h2r[d][:, :, BL * s:BL * (s + 1)],
                            in0=H2[d], in1=ones_cell[:, :, 0, :], op=ALU.mult)
                        if t % 8 == 0:
                            xt.pop((d, t - 8), None)

                # drain any leftover side work + tail feats blocks 0 and 7
                while side:
                    try:
                        next(side[0][1])
                    except StopIteration:
                        side.pop(0)
                for _, gen in featsq:
                    for _ in gen:
                        pass
                for gen in [feats_block(0), feats_block(7)]:
                    for _ in gen:
                        pass

            if DEBUG:
                for d in range(2):
                    nc.sync.dma_start(out=h2d[d], in_=hist2[d])

    nc.compile()
    return nc


def _prep_core_inputs(inputs):
    """Host-side: gather embeddings, fold scales, build per-core input maps."""
    emb_all = inputs["embed"][np.asarray(inputs["inputs"]).astype(np.int64)]
    emb_all = emb_all.astype(np.float32)          # (B, S, D)

    # tanh-trick + h*=2h folds:
    #  gates ACT computes tanh(0.5 z'); need z'_ifo = z_ifo, z'_g = 2 z_g
    #  h* = 2h absorbed: whh cols x0.5, attW x0.5, h2t x0.5
    w1s, whh1s, b1s, w2s, whh2s, b2s = [], [], [], [], [], []
    for d in range(2):
        w1s.append(_scale_gates(inputs["lstm1_wih"][d], 1.0, 2.0))
        whh1s.append(_scale_gates(inputs["lstm1_whh"][d] * 0.5, 1.0, 2.0))
        b1s.append(_scale_gates(inputs["lstm1_b"][d][:, None], 1.0, 2.0)[:, 0])
        w2s.append(_scale_gates(inputs["lstm2_wih"][d], 1.0, 2.0))
        whh2s.append(_scale_gates(inputs["lstm2_whh"][d] * 0.5, 1.0, 2.0))
        b2s.append(_scale_gates(inputs["lstm2_b"][d][:, None], 1.0, 2.0)[:, 0])

    w1 = np.stack([_lhsT_layout(w1s[d], 2) for d in range(2)], axis=1)
    whh1 = np.stack([_lhsT_layout(whh1s[d], 2) for d in range(2)], axis=1)
    w2 = np.stack([_lhsT_layout(w2s[d], 8) for d in range(2)], axis=1)
    whh2 = np.stack([_lhsT_layout(whh2s[d], 2) for d in range(2)], axis=1)

    # b1hl[hl, d, m, q] = hi/lo bf16 split of b1'[d, PERM-row m*128+q]
    import ml_dtypes
    b1m = np.stack([_reorder_rows(b1s[d][:, None])[:, 0].reshape(8, 128)
                    for d in range(2)])
    b1hi = b1m.astype(ml_dtypes.bfloat16)
    b1lo = (b1m - b1hi.astype(np.float32)).astype(ml_dtypes.bfloat16)
    b1hl = np.ascontiguousarray(
        np.stack([b1hi, b1lo]).transpose(2, 0, 1, 3))  # (8k, 2hl, 2d, 128)
    oneh = np.zeros((8, 8, P1 * BL), ml_dtypes.bfloat16)
    for k in range(8):
        oneh[k, k, :] = 1.0
    # b2col[p, d, m]
    b2col = np.stack([_reorder_rows(b2s[d][:, None])[:, 0].reshape(8, 128)
                      for d in range(2)])
    b2col = np.ascontiguousarray(b2col.transpose(2, 0, 1)).astype(np.float32)

    attW = inputs["attW"].astype(np.float64) * 0.5
    attWT = np.ascontiguousarray(
        attW.T.reshape(4, 128, 4, 128).transpose(1, 0, 2, 3)).astype(np.float32)

    h2t = inputs["h2t_w"].astype(np.float64) * 0.5    # (T, 512)
    h2tT = np.ascontiguousarray(
        h2t.T.reshape(2, 2, 128, T).transpose(2, 0, 1, 3)).astype(np.float32)
    h2tb = inputs["h2t_b"].astype(np.float32).reshape(T, 1)

    ident = np.eye(128, dtype=np.float32)

    shared = dict(w1T=w1, whh1T=whh1, w2T=w2, whh2T=whh2, b1hl=b1hl,
                  oneh=oneh, b2col=b2col, attWT=attWT, h2tT=h2tT, h2tb=h2tb,
                  ident=ident)

    in_maps = []
    for core in range(NCORES):
        bs = slice(core * BL, (core + 1) * BL)
        emb = emb_all[bs]                     # (BL, S, D)
        embp = np.zeros((BL, NV, D), np.float32)
        embp[:, D1:D1 + S] = emb
        # embP[p, c, v*BL + b] = embp[b, v, c*128+p]
        e = embp.transpose(2, 1, 0).reshape(2, 128, NV, BL)
        embPc = np.ascontiguousarray(
            e.transpose(1, 0, 2, 3).reshape(128, 2, NV * BL)).astype(np.float32)
        sent = inputs["sent_embs"][bs].astype(np.float32)
        sentTc = np.ascontiguousarray(
            sent.transpose(2, 0, 1).reshape(4, 128, BL, NS)
            .transpose(1, 2, 0, 3)).astype(np.float32)
        sentNc = np.ascontiguousarray(sent.transpose(1, 0, 2)).astype(np.float32)
        in_maps.append(dict(embP=embPc, sentT=sentTc, sentN=sentNc, **shared))
    return in_maps


def _viterbi_host(feats, trans):
    Bn, Sn, Tn = feats.shape
    fv = np.full((Bn, Tn), -10000.0, np.float32)
    fv[:, START] = 0.0
    bps = np.zeros((Bn, Sn, Tn), np.int32)
    for s in range(Sn):
        sc = fv[:, None, :] + trans[None, :, :]
        bps[:, s] = sc.argmax(-1)
        fv = sc.max(-1).astype(np.float32) + feats[:, s]
    term = fv + trans[STOP][None, :]
    tag = term.argmax(-1).astype(np.int32)
    path = np.zeros((Bn, Sn), np.int32)
    for s in range(Sn - 1, -1, -1):
        path[:, s] = tag
        tag = bps[np.arange(Bn), s, tag]
    return path


def _run(inputs, **spmd_kwargs):
    global _BUILT
    from concourse.bass_utils import run_bass_kernel_spmd

    inputs = {k: np.asarray(v) for k, v in inputs.items()}
    if _BUILT is None:
        _BUILT = _build()
    nc = _BUILT
    in_maps = _prep_core_inputs(inputs)
    return run_bass_kernel_spmd(nc, in_maps, core_ids=list(range(NCORES)),
                                **spmd_kwargs)


def kernel(**inputs):
    inputs = {k: np.asarray(v) for k, v in inputs.items()}
    in_dtype = inputs["inputs"].dtype
    res = _run(inputs)
    feats = np.zeros((B, S, T), np.float32)
    for core in range(NCORES):
        ft = res.results[core]["featsT"]      # (T, N_ALL), n = s*BL+b
        feats[core * BL:(core + 1) * BL] = (
            ft.reshape(T, S, BL).transpose(2, 1, 0))
    paths = _viterbi_host(feats, inputs["trans"].astype(np.float32))
    return paths.astype(in_dtype if np.issubdtype(in_dtype, np.integer)
                        else np.int32)


# revision 4
# speedup vs baseline: 1.0311x; 1.0311x over previous
"""Att-BiLSTM-CRF Trainium2 kernel, v2.

Data-parallel over batch: 32 seqs -> 8 cores x 4 (BL=4). All-f32 device math.

Device pipeline per core:
  scan1   chunked BiLSTM-1: P1=4 chunks/dir x (128+64) lockstep slots.
          Warmup slots (64) converge to the exact trajectory via forget-gate
          damping (validated: max |dh| <= 4e-4, zero Viterbi flips).
          Per slot/dir: x-part matmuls (W1@emb + bias K=1 ones-matmul)
          accumulate in PSUM, 16 Whh@h matmuls on top, ONE tanh gates-ACT
          (sigmoid via tanh(z/2), scales folded into weights host-side),
          3 fused scalar_tensor_tensor cell ops, h* = 2h carried (0.5
          absorbed into consumer weights), hist writes on gpsimd.
  phaseC  attention + X2 -> x2 gate preacts staged in DRAM, s-blocks of 64
          ordered {0,7,1,6,...} (edges first). Two blocks lead-in; the rest
          interleave into scan2's idle engine slots (emitted early enough in
          program order that every x2 write precedes its read).
  scan2   sequential 512 steps (layer-2 forget gates saturate; chunking
          provably does not converge). Same fused step structure; x2 enters
          PSUM via identity-matmul from 8-step DMA batches.
  phaseD  feats = h2t@l2 + bias, blocks interleaved middle-out into scan2.
host: embedding gather before; CRF Viterbi decode after (numpy).
"""

import numpy as np

S = 512
D = 256
H = 256
G = 4 * H
T = 12
NS = 16
B = 32
NCORES = 8
BL = B // NCORES
PAD, START, STOP = 0, 10, 11
N_ALL = S * BL

P1 = 4                    # scan1 chunks per direction
L1 = S // P1              # 128
D1 = 64                   # warmup slots
NSLOT = L1 + D1           # 192
NV = 640                  # padded virtual emb positions: [64 zero | 512 real | 64 zero]

NBLK = 8
BS = S // NBLK            # 64
BLK_ORDER = [0, 7, 1, 6, 2, 5, 3, 4]

PERM = [0, 1, 2, 3, 6, 7, 4, 5]  # m-tiles -> [i0,i1,f0,f1,o0,o1,g0,g1]

_BUILT = None
DEBUG = False
F32R_X2 = True
F32R_WX = False
F32R_FEATS = False


def _reorder_rows(w):
    wt = w.reshape(8, 128, -1)
    return wt[PERM].reshape(G, -1)


def _lhsT_layout(w, kchunks):
    """w: (G, K) -> (128, kchunks, 8, 128): [p,c,m,q] = w[m*128+q, c*128+p]."""
    wr = _reorder_rows(np.asarray(w, np.float32))
    K = wr.shape[1]
    assert K == kchunks * 128
    a = wr.T.reshape(kchunks, 128, 8, 128)
    return np.ascontiguousarray(a.transpose(1, 0, 2, 3)).astype(np.float32)


def _scale_gates(w, s_ifo, s_g):
    """Scale gate rows of (G, ...): i,f,o rows by s_ifo, g rows by s_g.
    Reference gate order along G is i,f,g,o (each H wide)."""
    w = np.asarray(w, np.float64).copy()
    w[0 * H:2 * H] *= s_ifo   # i, f
    w[2 * H:3 * H] *= s_g     # g
    w[3 * H:4 * H] *= s_ifo   # o
    return w.astype(np.float32)


def _build():
    import concourse.tile as tile
    from concourse.bacc import Bacc
    from concourse import mybir

    f32 = mybir.dt.float32
    f32r = mybir.dt.float32r
    AF = mybir.ActivationFunctionType
    ALU = mybir.AluOpType

    nc = Bacc()
    h1dt = f32r if F32R_WX else f32
    h2dt = f32r if F32R_FEATS else f32
    w2dt = f32r if F32R_X2 else f32

    def din(name, shape, dt=None):
        return nc.dram_tensor(name, shape, dt or f32, kind="ExternalInput")

    embP = din("embP", (128, 2, NV * BL))          # [p, kc, v*BL+b]
    w1T = din("w1T", (128, 2, 2, 8, 128))          # [p, dir, kc, m, q]
    whh1T = din("whh1T", (128, 2, 2, 8, 128))
    bf16 = mybir.dt.bfloat16
    b1hl = nc.dram_tensor("b1hl", (8, 2, 2, 128), bf16, kind="ExternalInput")
    oneh = nc.dram_tensor("oneh", (8, 8, P1 * BL), bf16, kind="ExternalInput")
    whh2T = din("whh2T", (128, 2, 2, 8, 128))
    w2T = din("w2T", (128, 2, 8, 8, 128), w2dt)
    b2col = din("b2col", (128, 2, 8))              # per-partition bias2 [p,d,m]
    attWT = din("attWT", (128, 4, 4, 128), h1dt)   # [p, kc, f, q] (x0.5 folded)
    sentT = din("sentT", (128, BL, 4, NS))
    sentN = din("sentN", (NS, BL, 2 * H))
    h2tT = din("h2tT", (128, 2, 2, T), h2dt)       # [p, dir, kc, t] (x0.5)
    h2tb = din("h2tb", (T, 1))
    identD = din("ident", (128, 128))

    scratch_kind = "ExternalOutput" if DEBUG else "Internal"
    x2d = nc.dram_tensor("x2d", (2, NBLK, 128, BS, 8, BL), f32,
                         kind=scratch_kind)
    if DEBUG:
        h1d = nc.dram_tensor("h1d", (2, 128, 2, N_ALL), h1dt,
                             kind="ExternalOutput")
        gtd = nc.dram_tensor("gtd", (128, 8, P1, BL), f32,
                             kind="ExternalOutput")
        ptd = nc.dram_tensor("ptd", (128, 8, P1, BL), f32,
                             kind="ExternalOutput")
        hsd = nc.dram_tensor("hsd", (128, 2, P1, BL), f32,
                             kind="ExternalOutput")
        wsd = nc.dram_tensor("wsd", (128, 2, P1, BL), f32,
                             kind="ExternalOutput")
        h2d = nc.dram_tensor("h2d", (2, 128, 2, N_ALL), h2dt,
                             kind="ExternalOutput")
    featsT = nc.dram_tensor("featsT", (T, N_ALL), f32, kind="ExternalOutput")

    with tile.TileContext(nc) as tc:
        with tc.tile_pool(name="persist", bufs=1) as pp:
            hist1 = [pp.tile([128, 2, N_ALL], h1dt, tag=f"h1_{d}", name=f"h1_{d}")
                     for d in range(2)]
            hist2 = [pp.tile([128, 2, N_ALL], h2dt, tag=f"h2_{d}", name=f"h2_{d}")
                     for d in range(2)]
            ident_sb = pp.tile([128, 128], f32)
            nc.sync.dma_start(out=ident_sb, in_=identD[:])
            b1hl_sb = pp.tile([8, 2, 2, 128], bf16)
            nc.sync.dma_start(out=b1hl_sb, in_=b1hl[:])
            oneh_sb = pp.tile([8, 8, P1 * BL], bf16)
            nc.sync.dma_start(out=oneh_sb, in_=oneh[:])
            ones16 = pp.tile([NS, NS], f32)
            nc.vector.memset(ones16, 1.0)
            ones_cell = pp.tile([128, 2, P1, BL], f32)
            nc.vector.memset(ones_cell, 1.0)
            ones_w = pp.tile([128, BS * BL], f32)
            nc.vector.memset(ones_w, 1.0)

            def hist_k(hist, c):
                return hist[0][:, c, :] if c < 2 else hist[1][:, c - 2, :]

            # ======================= scan1 (chunked) =======================
            with tc.tile_pool(name="s1w", bufs=1) as s1w, \
                 tc.tile_pool(name="s1st", bufs=3) as s1st, \
                 tc.tile_pool(name="s1ps", bufs=4, space="PSUM") as s1ps:
                emb_sb = s1w.tile([128, 2, NV * BL], f32, tag="embP", name="embP")
                nc.sync.dma_start(out=emb_sb, in_=embP[:])
                w1_sb = s1w.tile([128, 2, 2, 8, 128], f32, tag="w1", name="w1")
                nc.sync.dma_start(out=w1_sb, in_=w1T[:])
                whh1_sb = s1w.tile([128, 2, 2, 8, 128], f32, tag="wh1", name="wh1")
                nc.sync.dma_start(out=whh1_sb, in_=whh1T[:])

                embv = emb_sb.rearrange("p c (v b) -> p c v b", b=BL)
                Hs = [s1w.tile([128, 2, P1, BL], f32, tag=f"H{d}", name=f"H{d}")
                      for d in range(2)]
                Ws = [s1w.tile([128, 2, P1, BL], f32, tag=f"W{d}", name=f"W{d}")
                      for d in range(2)]
                for d in range(2):
                    nc.vector.memset(Hs[d], 0.0)
                    nc.vector.memset(Ws[d], 0.0)
                h1r = [hist1[d].rearrange("p c (q r) -> p c q r", q=P1)
                       for d in range(2)]

                for k in range(NSLOT):
                    if k == D1:
                        # chunk with no real history restarts from true init:
                        # fwd chunk 0 (seq start), bwd chunk P1-1 (seq end)
                        nc.vector.memset(Hs[0][:, :, 0, :], 0.0)
                        nc.vector.memset(Ws[0][:, :, 0, :], 0.0)
                        nc.vector.memset(Hs[1][:, :, P1 - 1, :], 0.0)
                        nc.vector.memset(Ws[1][:, :, P1 - 1, :], 0.0)
                    for d in range(2):
                        # fwd: chunk q reads v = q*128 + k  (pos = v - 64)
                        # bwd: chunk q reads v = q*128 + 255 - k
                        vbase = k if d == 0 else (255 - k)
                        pt = s1ps.tile([128, 8, P1, BL], f32, tag=f"pt{d}")
                        for m in range(8):
                            for c in range(2):
                                # start=True pends the whole 2KB zero region;
                                # each region's first touch then overwrites.
                                nc.tensor.matmul(
                                    pt[:, m, :, :], w1_sb[:, d, c, m, :],
                                    embv[:, c, vbase::L1, :][:, 0:P1, :],
                                    start=(m == 0 and c == 0), stop=False)
                        for hl in range(2):
                            nc.tensor.matmul(
                                pt.rearrange("p m q b -> p (m q b)"),
                                b1hl_sb[:, hl, d, :], oneh_sb.rearrange(
                                    "k m j -> k (m j)"),
                                start=False, stop=False)
                        for m in range(8):
                            for c in range(2):
                                nc.tensor.matmul(
                                    pt[:, m, :, :], whh1_sb[:, d, c, m, :],
                                    Hs[d][:, c, :, :],
                                    start=False, stop=(c == 1 and m == 7))
                        gt = s1st.tile([128, 8, P1, BL], f32, tag=f"g{d}")
                        if DEBUG and k == D1 and d == 0:
                            ptsb = s1st.tile([128, 8, P1, BL], f32, tag="ptdbg")
                            nc.vector.tensor_copy(ptsb, pt)
                            nc.sync.dma_start(out=ptd[:], in_=ptsb)
                        nc.scalar.activation(out=gt, in_=pt, func=AF.Tanh,
                                             scale=0.5)
                        if DEBUG and k == D1 and d == 0:
                            nc.sync.dma_start(out=gtd[:], in_=gt)
                        pv = s1st.tile([128, 2, P1, BL], f32, tag=f"p{d}")
                        nc.vector.scalar_tensor_tensor(
                            out=pv, in0=gt[:, 2:4], scalar=1.0, in1=Ws[d],
                            op0=ALU.add, op1=ALU.mult)
                        vv = s1st.tile([128, 2, P1, BL], f32, tag=f"v{d}")
                        nc.vector.scalar_tensor_tensor(
                            out=vv, in0=gt[:, 0:2], scalar=1.0, in1=gt[:, 6:8],
                            op0=ALU.add, op1=ALU.mult)
                        nc.vector.scalar_tensor_tensor(
                            out=Ws[d], in0=pv, scalar=0.5, in1=vv,
                            op0=ALU.mult, op1=ALU.add)
                        th = s1st.tile([128, 2, P1, BL], f32, tag=f"t{d}")
                        nc.scalar.activation(out=th, in_=Ws[d], func=AF.Tanh,
                                             scale=0.5)
                        nc.vector.scalar_tensor_tensor(
                            out=Hs[d], in0=gt[:, 4:6], scalar=1.0, in1=th,
                            op0=ALU.add, op1=ALU.mult)
                        if DEBUG and k == D1 and d == 0:
                            nc.sync.dma_start(out=hsd[:], in_=Hs[0])
                            nc.sync.dma_start(out=wsd[:], in_=Ws[0])
                        if k >= D1:
                            j = k - D1
                            col = j if d == 0 else (L1 - 1 - j)
                            nc.gpsimd.tensor_tensor(
                                out=h1r[d][:, :, :, BL * col:BL * (col + 1)],
                                in0=Hs[d], in1=ones_cell, op=ALU.mult)

            if DEBUG:
                for d in range(2):
                    nc.sync.dma_start(out=h1d[d], in_=hist1[d])

            # =================== phase C + scan2 + phase D ==================
            with tc.tile_pool(name="p2w", bufs=1) as p2w, \
                 tc.tile_pool(name="p2c", bufs=2) as p2c, \
                 tc.tile_pool(name="p2s", bufs=3) as p2s, \
                 tc.tile_pool(name="x2p", bufs=2) as x2p, \
                 tc.tile_pool(name="stgp", bufs=1) as stgp, \
                 tc.tile_pool(name="cmbp", bufs=1) as cmbp, \
                 tc.tile_pool(name="aps", bufs=2, space="PSUM") as aps, \
                 tc.tile_pool(name="s2ps", bufs=2, space="PSUM") as s2ps:
                attW_sb = p2w.tile([128, 4, 4, 128], h1dt, tag="aW", name="aW")
                nc.sync.dma_start(out=attW_sb, in_=attWT[:])
                sentT_sb = p2w.tile([128, BL, 4, NS], f32, tag="sT", name="sT")
                nc.sync.dma_start(out=sentT_sb, in_=sentT[:])
                sentN_sb = p2w.tile([NS, BL, 2 * H], f32, tag="sN", name="sN")
                nc.sync.dma_start(out=sentN_sb, in_=sentN[:])
                w2_sb = p2w.tile([128, 2, 8, 8, 128], w2dt, tag="w2", name="w2")
                nc.sync.dma_start(out=w2_sb, in_=w2T[:])
                whh2_sb = p2w.tile([128, 2, 2, 8, 128], f32, tag="wh2", name="wh2")
                nc.sync.dma_start(out=whh2_sb, in_=whh2T[:])
                b2_sb = p2w.tile([128, 2, 8], f32, tag="b2", name="b2")
                nc.sync.dma_start(out=b2_sb, in_=b2col[:])
                h2t_sb = p2w.tile([128, 2, 2, T], h2dt, tag="h2t", name="h2t")
                nc.sync.dma_start(out=h2t_sb, in_=h2tT[:])
                h2tb_sb = p2w.tile([T, 1], f32, tag="h2tb", name="h2tb")
                nc.sync.dma_start(out=h2tb_sb, in_=h2tb[:])

                def phasec_block(pieces):
                    """Attention + X2 for a list of (blk, s0, ns) pieces
                    (total width 64 positions = 256 n, so the X2 matmuls hit
                    the f32r fast-rate threshold). Yields per ~PE quantum."""
                    assert sum(ns for _, _, ns in pieces) == BS
                    cmb = cmbp.tile([128, 4, BS * BL], f32, tag="cmb")
                    cmbR = cmbp.tile([128, 8, BS * BL],
                                     f32r if F32R_X2 else f32, tag="cmbR",
                                     name="cmbR")
                    cmbv = cmb.rearrange("p f (s b) -> p f s b", b=BL)
                    cmbRv = cmbR.rearrange("p f (s b) -> p f s b", b=BL)
                    col = 0
                    for blk, s0, ns in pieces:
                        n0 = (blk * BS + s0) * BL
                        NW = ns * BL
                        c0 = col * BL
                        for f in range(4):
                            pwx_t = aps.tile([128, BS * BL], f32, tag="big")
                            pwx = pwx_t[:, 0:NW]
                            for c in range(4):
                                nc.tensor.matmul(
                                    pwx, attW_sb[:, c, f, :],
                                    hist_k(hist1, c)[:, n0:n0 + NW],
                                    start=(c == 0), stop=(c == 3))
                                if c == 1 and ns > 16:
                                    yield
                            nc.vector.tensor_copy(cmb[:, f, c0:c0 + NW], pwx)
                            nc.gpsimd.tensor_tensor(
                                out=cmbR[:, f, c0:c0 + NW],
                                in0=cmb[:, f, c0:c0 + NW],
                                in1=ones_w[:, 0:NW], op=ALU.mult)
                            yield
                        for b in range(BL):
                            ps16 = aps.tile([128, BS], f32, tag="sml")
                            for c in range(4):
                                nc.tensor.matmul(
                                    ps16[0:NS, 0:ns], sentT_sb[:, b, c, :],
                                    cmbv[:, c, col:col + ns, b],
                                    start=(c == 0), stop=(c == 3))
                            et = p2c.tile([NS, BS], f32, tag="et")
                            nc.scalar.activation(out=et[:, 0:ns],
                                                 in_=ps16[0:NS, 0:ns],
                                                 func=AF.Exp)
                            psum = aps.tile([128, BS], f32, tag="sml")
                            nc.tensor.matmul(psum[0:NS, 0:ns], ones16,
                                             et[:, 0:ns], start=True, stop=True)
                            rs = p2c.tile([NS, BS], f32, tag="rs")
                            nc.vector.reciprocal(out=rs[:, 0:ns],
                                                 in_=psum[0:NS, 0:ns])
                            aw = p2c.tile([NS, BS], f32, tag="aw")
                            nc.vector.tensor_mul(out=aw[:, 0:ns],
                                                 in0=et[:, 0:ns],
                                                 in1=rs[:, 0:ns])
                            yield
                            for f in range(4):
                                pg = aps.tile([128, BS], f32, tag="sml")
                                nc.tensor.matmul(
                                    pg[:, 0:ns],
                                    sentN_sb[:, b, 128 * f:128 * (f + 1)],
                                    aw[:, 0:ns], start=True, stop=True)
                                nc.vector.tensor_copy(
                                    cmbRv[:, 4 + f, col:col + ns, b],
                                    pg[:, 0:ns])
                                if f % 2 == 1 and ns > 16:
                                    yield
                        col += ns
                    for d in range(2):
                        stg = stgp.tile([128, BS, 8, BL], f32, tag=f"sg{d}")
                        for m in range(8):
                            px_t = aps.tile([128, BS * BL], f32, tag="big")
                            px = px_t
                            for c in range(8):
                                nc.tensor.matmul(
                                    px, w2_sb[:, d, c, m, :], cmbR[:, c, :],
                                    start=(c == 0), stop=(c == 7))
                                if c % 2 == 1 and not F32R_X2:
                                    yield
                            nc.scalar.activation(
                                out=stg[:, :, m, :],
                                in_=px.rearrange("p (s b) -> p s b", b=BL),
                                func=AF.Identity,
                                bias=b2_sb[:, d, m:m + 1])
                            yield
                        col = 0
                        for blk, s0, ns in pieces:
                            nc.sync.dma_start(
                                out=x2d[d, blk, :, s0:s0 + ns],
                                in_=stg[:, col:col + ns])
                            col += ns
                        yield

                def feats_block(blk):
                    n0 = blk * BS * BL
                    pf = aps.tile([128, BS * BL], f32, tag="big")
                    first = True
                    for d in range(2):
                        for c in range(2):
                            nc.tensor.matmul(
                                pf[0:T, :], h2t_sb[:, d, c, :],
                                hist2[d][:, c, n0:n0 + BS * BL],
                                start=first, stop=(d == 1 and c == 1))
                            first = False
                    yield
                    ft = p2c.tile([T, BS * BL], f32, tag="ft")
                    nc.vector.tensor_scalar_add(out=ft, in0=pf[0:T, :],
                                                scalar1=h2tb_sb)
                    nc.sync.dma_start(out=featsT[:, n0:n0 + BS * BL], in_=ft)
                    yield

                # lead-in: first 32 positions of each sequence end
                for _ in phasec_block([(0, 0, 16), (7, 48, 16),
                                       (0, 16, 16), (7, 32, 16)]):
                    pass

                # remaining phase C, deadline-paced into scan2 (deadline =
                # scan2 step by which the group must be fully EMITTED --
                # its x2 read DMA is issued at consumption_step - 8)
                side = []
                for dl, pieces in (
                        (22, [(0, 32, 16), (7, 16, 16),
                              (0, 48, 16), (7, 0, 16)]),
                        (54, [(1, 0, 32), (6, 32, 32)]),
                        (86, [(1, 32, 32), (6, 0, 32)]),
                        (118, [(2, 0, 64)]), (120, [(5, 0, 64)]),
                        (182, [(3, 0, 64)]), (184, [(4, 0, 64)])):
                    side.append((dl, phasec_block(pieces)))
                featsq = [(322, feats_block(3)), (324, feats_block(4)),
                          (386, feats_block(2)), (388, feats_block(5)),
                          (450, feats_block(1)), (452, feats_block(6))]

                def emit_side(t):
                    while side:
                        dl = side[0][0]
                        if dl <= t:
                            quota = 1000   # overdue: flush
                        elif dl - t < 16:
                            quota = 8
                        else:
                            quota = 4
                        for _ in range(quota):
                            try:
                                next(side[0][1])
                            except StopIteration:
                                side.pop(0)
                                break
                        else:
                            break
                        if not side or side[0][0] > t + 16:
                            break
                    while featsq and featsq[0][0] <= t:
                        _, gen = featsq[0]
                        done = True
                        for _ in gen:
                            done = False
                            break
                        if done:
                            featsq.pop(0)

                # ----------------------- scan2 ------------------------------
                H2 = [p2w.tile([128, 2, BL], f32, tag=f"H2{d}", name=f"H2{d}")
                      for d in range(2)]
                W2 = [p2w.tile([128, 2, BL], f32, tag=f"W2{d}", name=f"W2{d}")
                      for d in range(2)]
                for d in range(2):
                    nc.vector.memset(H2[d], 0.0)
                    nc.vector.memset(W2[d], 0.0)
                h2r = [hist2[d] for d in range(2)]
                xt = {}

                def load_batch(d, tb):
                    """DMA the 8-step x2 batch covering steps tb..tb+7."""
                    if d == 0:
                        blk, s0 = tb // BS, tb % BS
                    else:
                        s = S - 1 - tb
                        blk, s0 = s // BS, (s % BS) - 7
                    tl = x2p.tile([128, 8, 8, BL], f32, tag=f"x{d}")
                    nc.sync.dma_start(out=tl, in_=x2d[d, blk, :, s0:s0 + 8])
                    return tl

                for d in range(2):
                    xt[(d, 0)] = load_batch(d, 0)

                for t in range(S):
                    if t % 8 == 0:
                        emit_side(t)
                        if t + 8 < S:
                            for d in range(2):
                                xt[(d, t + 8)] = load_batch(d, t + 8)
                    for d in range(2):
                        tb = t - t % 8
                        idx = (t % 8) if d == 0 else (7 - t % 8)
                        xtile = xt[(d, tb)]
                        pt = s2ps.tile([128, 8, BL], f32, tag=f"q{d}")
                        nc.tensor.matmul(
                            pt, ident_sb, xtile[:, idx, :, :],
                            start=True, stop=False)
                        for m in range(8):
                            for c in range(2):
                                nc.tensor.matmul(
                                    pt[:, m, :], whh2_sb[:, d, c, m, :],
                                    H2[d][:, c, :],
                                    start=False, stop=(c == 1 and m == 7))
                        gt = p2s.tile([128, 8, BL], f32, tag=f"G{d}")
                        nc.scalar.activation(out=gt, in_=pt, func=AF.Tanh,
                                             scale=0.5)
                        pv = p2s.tile([128, 2, BL], f32, tag=f"P{d}")
                        nc.vector.scalar_tensor_tensor(
                            out=pv, in0=gt[:, 2:4], scalar=1.0, in1=W2[d],
                            op0=ALU.add, op1=ALU.mult)
                        vv = p2s.tile([128, 2, BL], f32, tag=f"V{d}")
                        nc.vector.scalar_tensor_tensor(
                            out=vv, in0=gt[:, 0:2], scalar=1.0, in1=gt[:, 6:8],
                            op0=ALU.add, op1=ALU.mult)
                        nc.vector.scalar_tensor_tensor(
                            out=W2[d], in0=pv, scalar=0.5, in1=vv,
                            op0=ALU.mult, op1=ALU.add)
                        th = p2s.tile([128, 2, BL], f32, tag=f"T{d}")
                        nc.scalar.activation(out=th, in_=W2[d], func=AF.Tanh,
                                             scale=0.5)
                        nc.vector.scalar_tensor_tensor(
                            out=H2[d], in0=gt[:, 4:6], scalar=1.0, in1=th,
                            op0=ALU.add, op1=ALU.mult)
                        s = t if d == 0 else S - 1 - t
                        nc.gpsimd.tensor_tensor(
                            out=h2r[d][:, :, BL * s:BL * (s + 1)],
                            in0=H2[d], in1=ones_cell[:, :, 0, :], op=ALU.mult)
                        if t % 8 == 0:
                            xt.pop((d, t - 8), None)
                    if t % 8 == 4:
                        emit_side(t)

                # drain any leftover side work + tail feats blocks 0 and 7
                while side:
                    try:
                        next(side[0][1])
                    except StopIteration:
                        side.pop(0)
                for _, gen in featsq:
                    for _ in gen:
                        pass
                for gen in [feats_block(0), feats_block(7)]:
                    for _ in gen:
                        pass

            if DEBUG:
                for d in range(2):
                    nc.sync.dma_start(out=h2d[d], in_=hist2[d])

    nc.compile()
    return nc


def _prep_core_inputs(inputs):
    """Host-side: gather embeddings, fold scales, build per-core input maps."""
    emb_all = inputs["embed"][np.asarray(inputs["inputs"]).astype(np.int64)]
    emb_all = emb_all.astype(np.float32)          # (B, S, D)

    # tanh-trick + h*=2h folds:
    #  gates ACT computes tanh(0.5 z'); need z'_ifo = z_ifo, z'_g = 2 z_g
    #  h* = 2h absorbed: whh cols x0.5, attW x0.5, h2t x0.5
    w1s, whh1s, b1s, w2s, whh2s, b2s = [], [], [], [], [], []
    for d in range(2):
        w1s.append(_scale_gates(inputs["lstm1_wih"][d], 1.0, 2.0))
        whh1s.append(_scale_gates(inputs["lstm1_whh"][d] * 0.5, 1.0, 2.0))
        b1s.append(_scale_gates(inputs["lstm1_b"][d][:, None], 1.0, 2.0)[:, 0])
        w2s.append(_scale_gates(inputs["lstm2_wih"][d], 1.0, 2.0))
        whh2s.append(_scale_gates(inputs["lstm2_whh"][d] * 0.5, 1.0, 2.0))
        b2s.append(_scale_gates(inputs["lstm2_b"][d][:, None], 1.0, 2.0)[:, 0])

    w1 = np.stack([_lhsT_layout(w1s[d], 2) for d in range(2)], axis=1)
    whh1 = np.stack([_lhsT_layout(whh1s[d], 2) for d in range(2)], axis=1)
    w2 = np.stack([_lhsT_layout(w2s[d], 8) for d in range(2)], axis=1)
    whh2 = np.stack([_lhsT_layout(whh2s[d], 2) for d in range(2)], axis=1)

    # b1hl[hl, d, m, q] = hi/lo bf16 split of b1'[d, PERM-row m*128+q]
    import ml_dtypes
    b1m = np.stack([_reorder_rows(b1s[d][:, None])[:, 0].reshape(8, 128)
                    for d in range(2)])
    b1hi = b1m.astype(ml_dtypes.bfloat16)
    b1lo = (b1m - b1hi.astype(np.float32)).astype(ml_dtypes.bfloat16)
    b1hl = np.ascontiguousarray(
        np.stack([b1hi, b1lo]).transpose(2, 0, 1, 3))  # (8k, 2hl, 2d, 128)
    oneh = np.zeros((8, 8, P1 * BL), ml_dtypes.bfloat16)
    for k in range(8):
        oneh[k, k, :] = 1.0
    # b2col[p, d, m]
    b2col = np.stack([_reorder_rows(b2s[d][:, None])[:, 0].reshape(8, 128)
                      for d in range(2)])
    b2col = np.ascontiguousarray(b2col.transpose(2, 0, 1)).astype(np.float32)

    attW = inputs["attW"].astype(np.float64) * 0.5
    attWT = np.ascontiguousarray(
        attW.T.reshape(4, 128, 4, 128).transpose(1, 0, 2, 3)).astype(np.float32)

    h2t = inputs["h2t_w"].astype(np.float64) * 0.5    # (T, 512)
    h2tT = np.ascontiguousarray(
        h2t.T.reshape(2, 2, 128, T).transpose(2, 0, 1, 3)).astype(np.float32)
    h2tb = inputs["h2t_b"].astype(np.float32).reshape(T, 1)

    ident = np.eye(128, dtype=np.float32)

    shared = dict(w1T=w1, whh1T=whh1, w2T=w2, whh2T=whh2, b1hl=b1hl,
                  oneh=oneh, b2col=b2col, attWT=attWT, h2tT=h2tT, h2tb=h2tb,
                  ident=ident)

    in_maps = []
    for core in range(NCORES):
        bs = slice(core * BL, (core + 1) * BL)
        emb = emb_all[bs]                     # (BL, S, D)
        embp = np.zeros((BL, NV, D), np.float32)
        embp[:, D1:D1 + S] = emb
        # embP[p, c, v*BL + b] = embp[b, v, c*128+p]
        e = embp.transpose(2, 1, 0).reshape(2, 128, NV, BL)
        embPc = np.ascontiguousarray(
            e.transpose(1, 0, 2, 3).reshape(128, 2, NV * BL)).astype(np.float32)
        sent = inputs["sent_embs"][bs].astype(np.float32)
        sentTc = np.ascontiguousarray(
            sent.transpose(2, 0, 1).reshape(4, 128, BL, NS)
            .transpose(1, 2, 0, 3)).astype(np.float32)
        sentNc = np.ascontiguousarray(sent.transpose(1, 0, 2)).astype(np.float32)
        in_maps.append(dict(embP=embPc, sentT=sentTc, sentN=sentNc, **shared))
    return in_maps


def _viterbi_host(feats, trans):
    Bn, Sn, Tn = feats.shape
    fv = np.full((Bn, Tn), -10000.0, np.float32)
    fv[:, START] = 0.0
    bps = np.zeros((Bn, Sn, Tn), np.int32)
    for s in range(Sn):
        sc = fv[:, None, :] + trans[None, :, :]
        bps[:, s] = sc.argmax(-1)
        fv = sc.max(-1).astype(np.float32) + feats[:, s]
    term = fv + trans[STOP][None, :]
    tag = term.argmax(-1).astype(np.int32)
    path = np.zeros((Bn, Sn), np.int32)
    for s in range(Sn - 1, -1, -1):
        path[:, s] = tag
        tag = bps[np.arange(Bn), s, tag]
    return path


def _run(inputs, **spmd_kwargs):
    global _BUILT
    from concourse.bass_utils import run_bass_kernel_spmd

    inputs = {k: np.asarray(v) for k, v in inputs.items()}
    if _BUILT is None:
        _BUILT = _build()
    nc = _BUILT
    in_maps = _prep_core_inputs(inputs)
    return run_bass_kernel_spmd(nc, in_maps, core_ids=list(range(NCORES)),
                                **spmd_kwargs)


def kernel(**inputs):
    inputs = {k: np.asarray(v) for k, v in inputs.items()}
    in_dtype = inputs["inputs"].dtype
    res = _run(inputs)
    feats = np.zeros((B, S, T), np.float32)
    for core in range(NCORES):
        ft = res.results[core]["featsT"]      # (T, N_ALL), n = s*BL+b
        feats[core * BL:(core + 1) * BL] = (
            ft.reshape(T, S, BL).transpose(2, 1, 0))
    paths = _viterbi_host(feats, inputs["trans"].astype(np.float32))
    return paths.astype(in_dtype if np.issubdtype(in_dtype, np.integer)
                        else np.int32)
